# revision 1
# baseline (speedup 1.0000x reference)
"""4-layer LSTM decoder (nn_Decoder) on 8 Trainium2 NeuronCores.

Sharding: model-parallel over the gate/hidden dimension (each core owns 128
of the 1024 hidden units of every layer, i.e. 512 of the 4096 gate rows).
The sequential recurrence is scheduled as a wavefront over anti-diagonals
d = t + layer; each diagonal does all its gate GEMMs (float32r, full speed),
the LSTM cells, then ONE 8-rank AllGather of the transposed hidden-state
slices so every core has the full h needed by the next diagonal.
"""
import sys
sys.path.insert(0, '/opt/trn_rl_repo')
import numpy as np

NLAYERS, NHID, NOUT, BSZ, STEPS = 4, 1024, 512, 64, 128
NC = 8           # cores
HS = NHID // NC  # 128 h-units per core
GS = 4 * HS      # 512 gate rows per core (i|f|o|c blocks of 128)
NOS = NOUT // NC  # 64 output cols per core
KCH = NHID // 128  # 8 contraction chunks

_RUNNER_CACHE = {}


def _build(steps, no_cc=False, no_tp=False, no_act=False, no_unpack=False):
    import concourse.bass as bass
    import concourse.bacc as bacc
    import concourse.mybir as mybir
    from concourse.tile import TileContext

    f32, f32r = mybir.dt.float32, mybir.dt.float32r
    AF = mybir.ActivationFunctionType

    nc = bacc.Bacc(name="lstm_dec")
    # inputs (per-core slices prepared on host)
    h0_d = nc.dram_tensor("h0", [128, NLAYERS * 512], f32, kind="ExternalInput")
    c0_d = nc.dram_tensor("c0", [64, NLAYERS * HS], f32, kind="ExternalInput")
    ut_d = nc.dram_tensor("ut", [128, NLAYERS * KCH * GS], f32, kind="ExternalInput")
    wt_d = nc.dram_tensor("wt", [128, (NLAYERS - 1) * KCH * GS], f32, kind="ExternalInput")
    lt_d = nc.dram_tensor("lt", [128, KCH * NOS], f32, kind="ExternalInput")
    id_d = nc.dram_tensor("id64", [64, 64], f32, kind="ExternalInput")
    out_d = nc.dram_tensor("out", [steps, 64, NOS], f32, kind="ExternalOutput")

    ndiag = steps + 3  # diagonals 1..ndiag; cells (t,i): t=d-i in [1, steps-1]
    tmax = steps - 1

    with TileContext(nc) as tc:
        with (
            tc.tile_pool(name="wpool", bufs=1) as wpool,
            tc.tile_pool(name="stage", bufs=2) as stage,
            tc.tile_pool(name="ht", bufs=3) as htp,
            tc.tile_pool(name="ct", bufs=2) as ctp,
            tc.tile_pool(name="tmp", bufs=8) as tmp,
            tc.tile_pool(name="io", bufs=4) as iop,
            tc.tile_pool(name="ps", bufs=4, space="PSUM") as pp,
            tc.tile_pool(name="pst", bufs=2, space="PSUM") as ppt,
            tc.tile_pool(name="psl", bufs=2, space="PSUM") as ppl,
            tc.tile_pool(name="dram", bufs=4, space="DRAM") as dram,
        ):
            # --- load + round weights to f32r ---
            ut = wpool.tile([128, NLAYERS * KCH * GS], f32r, tag="ut")
            wt = wpool.tile([128, (NLAYERS - 1) * KCH * GS], f32r, tag="wt")
            lt = wpool.tile([128, KCH * NOS], f32r, tag="lt")
            CH = 2048
            for dst, src_t, width in ((ut, ut_d, NLAYERS * KCH * GS),
                                      (wt, wt_d, (NLAYERS - 1) * KCH * GS),
                                      (lt, lt_d, KCH * NOS)):
                for off in range(0, width, CH):
                    w = min(CH, width - off)
                    st = stage.tile([128, CH], f32, tag="stage")
                    nc.sync.dma_start(st[:, :w], src_t[:, off:off + w])
                    nc.vector.tensor_copy(dst[:, off:off + w], st[:, :w])
            ident = wpool.tile([64, 64], f32, tag="ident")
            nc.sync.dma_start(ident[:], id_d[:])
            h0sb = wpool.tile([128, NLAYERS * 512], f32, tag="h0sb")
            nc.sync.dma_start(h0sb[:], h0_d[:])
            ht_init = htp.tile([128, NLAYERS * 512], f32r, tag="ht")
            nc.vector.tensor_copy(ht_init[:], h0sb[:])
            ct_init = ctp.tile([64, NLAYERS * HS], f32, tag="ct")
            nc.sync.dma_start(ct_init[:], c0_d[:])

            ht_read, ct_read = ht_init, ct_init

            for d in range(1, ndiag + 1):
                cells = [(d - i, i) for i in range(NLAYERS) if 1 <= d - i <= tmax]
                # --- gate GEMMs: psum_i = h(t-1)@U_i.T + hy_{i-1}(t)@W_{i-1}.T ---
                psums = {}
                for (t, i) in cells:
                    ps = pp.tile([64, GS], f32, tag="gates")
                    psums[i] = ps
                    last_u = (i == 0)
                    for ch in range(KCH):
                        nc.tensor.matmul(
                            ps[:], ht_read[:, ch * 256 + i * 64: ch * 256 + i * 64 + 64],
                            ut[:, (i * KCH + ch) * GS: (i * KCH + ch + 1) * GS],
                            start=(ch == 0), stop=(last_u and ch == KCH - 1))
                    if i > 0:
                        j = i - 1
                        for ch in range(KCH):
                            nc.tensor.matmul(
                                ps[:], ht_read[:, ch * 256 + j * 64: ch * 256 + j * 64 + 64],
                                wt[:, (j * KCH + ch) * GS: (j * KCH + ch + 1) * GS],
                                start=False, stop=(ch == KCH - 1))

                # --- L projection for t_L (needs gathered h3 of t_L) ---
                t_L = 0 if d == 1 else (d - 4 if 5 <= d <= ndiag else None)
                if t_L is not None:
                    psl = ppl.tile([64, NOS], f32, tag="lproj")
                    for ch in range(KCH):
                        nc.tensor.matmul(
                            psl[:], ht_read[:, ch * 256 + 3 * 64: ch * 256 + 3 * 64 + 64],
                            lt[:, ch * NOS:(ch + 1) * NOS],
                            start=(ch == 0), stop=(ch == KCH - 1))
                    so = iop.tile([64, NOS], f32, tag="so")
                    nc.vector.tensor_copy(so[:], psl[:])
                    nc.sync.dma_start(out_d[t_L, :, :], so[:])

                # --- cells ---
                ct_new, tp = None, None
                if cells:
                    ct_new = ctp.tile([64, NLAYERS * HS], f32, tag="ct")
                    tp = ppt.tile([128, NLAYERS * 64], f32, tag="tpose")
                sifos, tccs, tcys, hys = {}, {}, {}, {}
                for (t, i) in cells:
                    sifo = tmp.tile([64, 384], f32, tag="sifo")
                    sifos[i] = sifo
                    nc.scalar.activation(sifo[:], psums[i][:, :384], AF.Sigmoid)
                for (t, i) in cells:
                    tcc = tmp.tile([64, HS], f32, tag="tcc")
                    tccs[i] = tcc
                    nc.scalar.activation(tcc[:], psums[i][:, 384:512], AF.Tanh)
                for (t, i) in cells:
                    m1 = tmp.tile([64, HS], f32, tag="m1")
                    nc.vector.tensor_mul(m1[:], sifos[i][:, 128:256], ct_read[:, i * HS:(i + 1) * HS])
                    m2 = tmp.tile([64, HS], f32, tag="m2")
                    nc.vector.tensor_mul(m2[:], sifos[i][:, 0:128], tccs[i][:])
                    nc.vector.tensor_add(ct_new[:, i * HS:(i + 1) * HS], m1[:], m2[:])
                for (t, i) in cells:
                    tcy = tmp.tile([64, HS], f32, tag="tcy")
                    tcys[i] = tcy
                    nc.scalar.activation(tcy[:], ct_new[:, i * HS:(i + 1) * HS], AF.Tanh)
                for (t, i) in cells:
                    hy = tmp.tile([64, HS], f32, tag="hy")
                    nc.vector.tensor_mul(hy[:], sifos[i][:, 256:384], tcys[i][:])
                    nc.tensor.transpose(tp[:, i * 64:(i + 1) * 64], hy[:], ident[:])
                # carry c for layers not yet started (early diagonals only)
                if d <= NLAYERS:
                    started = {i for (_, i) in cells}
                    for i in range(NLAYERS):
                        if i not in started and d <= i:
                            nc.vector.tensor_copy(
                                ct_new[:, i * HS:(i + 1) * HS], ct_read[:, i * HS:(i + 1) * HS])

                # --- pack contribution + AllGather + unpack ---
                if d <= ndiag - 1 and cells:
                    contrib = iop.tile([128, NLAYERS * 64], f32r, tag="contrib")
                    cell_layers = {i for (_, i) in cells}
                    if len(cell_layers) == NLAYERS:
                        nc.vector.tensor_copy(contrib[:], tp[:])
                    else:
                        for i in range(NLAYERS):
                            if i in cell_layers:
                                nc.vector.tensor_copy(contrib[:, i * 64:(i + 1) * 64],
                                                      tp[:, i * 64:(i + 1) * 64])
                            else:
                                nc.vector.tensor_scalar_mul(
                                    contrib[:, i * 64:(i + 1) * 64],
                                    h0sb[:, i * 512:i * 512 + 64], 0.0)
                    cc_in = dram.tile([128, NLAYERS * 64], f32, tag="cc_in")
                    cc_out = dram.tile([NC * 128, NLAYERS * 64], f32, tag="cc_out")
                    nc.sync.dma_start(cc_in[:], contrib[:].bitcast(f32))
                    if no_cc:
                        for _r in range(NC):
                            nc.sync.dma_start(cc_out[_r * 128:(_r + 1) * 128, :], cc_in[:])
                    else:
                        nc.gpsimd.collective_compute(
                            "AllGather", mybir.AluOpType.bypass,
                            replica_groups=[list(range(NC))],
                            ins=[cc_in[:].opt()], outs=[cc_out[:].opt()])
                    ht_new = htp.tile([128, NLAYERS * 512], f32r, tag="ht")
                    for r in range(NC):
                        nc.sync.dma_start(ht_new[:, r * 256:(r + 1) * 256],
                                          cc_out[r * 128:(r + 1) * 128, :].bitcast(f32r))
                    for i in range(NLAYERS):
                        if d - i < 1:  # layer not started: fill slot locally from init
                            dstv = ht_new[:].rearrange("p (r l b) -> p r l b", r=NC, l=NLAYERS, b=64)[:, :, i, :]
                            srcv = h0sb[:].rearrange("p (r l b) -> p r l b", r=NC, l=NLAYERS, b=64)[:, :, i, :]
                            nc.vector.tensor_copy(dstv, srcv)
                    ht_read = ht_new
                if cells:
                    ct_read = ct_new
    nc.finalize()
    return nc


def _prep_inputs(hx, cx, W, U, L):
    hx = np.asarray(hx, np.float32)
    cx = np.asarray(cx, np.float32)
    W = np.asarray(W, np.float32)
    U = np.asarray(U, np.float32)
    L = np.asarray(L, np.float32)
    # gathered-h layout (rank-major): h0[p, r*256 + l*64 + b] = hx[l, b, r*128+p]
    h0 = hx.transpose(0, 2, 1).reshape(NLAYERS, NC, 128, 64)  # [l, r, p, b]
    h0 = h0.transpose(2, 1, 0, 3).reshape(128, NLAYERS * 512)
    h0 = np.ascontiguousarray(h0)

    ident = np.eye(64, dtype=np.float32)

    ins = []
    for k in range(NC):
        # local gate rows in [i|f|o|c] order: global U/W rows
        rows = np.concatenate([
            np.arange(0 * NHID + k * HS, 0 * NHID + (k + 1) * HS),   # i
            np.arange(1 * NHID + k * HS, 1 * NHID + (k + 1) * HS),   # f
            np.arange(3 * NHID + k * HS, 3 * NHID + (k + 1) * HS),   # o
            np.arange(2 * NHID + k * HS, 2 * NHID + (k + 1) * HS),   # c
        ])
        # ut[p, (l*KCH+ch)*GS + n] = U[l, rows[n], ch*128+p]
        Usl = U[:, rows, :]                       # [l, 512, 1024]
        ut = Usl.reshape(NLAYERS, GS, KCH, 128).transpose(3, 0, 2, 1)  # [p, l, ch, n]
        ut = np.ascontiguousarray(ut.reshape(128, NLAYERS * KCH * GS))
        Wsl = W[:, rows, :]                       # [3, 512, 1024]
        wtk = Wsl.reshape(NLAYERS - 1, GS, KCH, 128).transpose(3, 0, 2, 1)
        wtk = np.ascontiguousarray(wtk.reshape(128, (NLAYERS - 1) * KCH * GS))
        # lt[p, ch*NOS + j] = L[k*NOS + j, ch*128 + p]
        Lsl = L[k * NOS:(k + 1) * NOS, :]         # [64, 1024]
        ltk = Lsl.reshape(NOS, KCH, 128).transpose(2, 1, 0)
        ltk = np.ascontiguousarray(ltk.reshape(128, KCH * NOS))
        # c0[b, l*HS + j] = cx[l, b, k*HS + j]
        c0 = cx[:, :, k * HS:(k + 1) * HS].transpose(1, 0, 2)
        c0 = np.ascontiguousarray(c0.reshape(64, NLAYERS * HS))
        ins.append({"h0": h0, "c0": c0, "ut": ut, "wt": wtk, "lt": ltk, "id64": ident})
    return ins


class _Runner:
    def __init__(self, nc, n_cores=NC, donate=True):
        import jax
        from jax.sharding import Mesh, PartitionSpec
        from jax.experimental.shard_map import shard_map
        from concourse import bass2jax, mybir
        bass2jax.install_neuronx_cc_hook()
        self.n_cores = n_cores
        partition_name = nc.partition_id_tensor.name if nc.partition_id_tensor else None
        in_names, out_names, out_avals, zero_outs = [], [], [], []
        for alloc in nc.m.functions[0].allocations:
            if not isinstance(alloc, mybir.MemoryLocationSet):
                continue
            name = alloc.memorylocations[0].name
            if alloc.kind == "ExternalInput":
                if name != partition_name:
                    in_names.append(name)
            elif alloc.kind == "ExternalOutput":
                out_names.append(name)
                shape = tuple(alloc.tensor_shape)
                dtype = mybir.dt.np(alloc.dtype)
                out_avals.append(jax.core.ShapedArray(shape, dtype))
                zero_outs.append(np.zeros(shape, dtype))
        self.in_names, self.out_names = in_names, out_names
        self.out_avals, self.zero_outs = out_avals, zero_outs
        n_params = len(in_names)
        self.n_params = n_params
        all_in_names = in_names + out_names
        if partition_name is not None:
            all_in_names.append(partition_name)
        donate_idx = tuple(range(n_params, n_params + len(out_avals))) if donate else ()

        def _body(*args):
            operands = list(args)
            if partition_name is not None:
                operands.append(bass2jax.partition_id_tensor())
            outs = bass2jax._bass_exec_p.bind(
                *operands, out_avals=tuple(out_avals), in_names=tuple(all_in_names),
                out_names=tuple(out_names), lowering_input_output_aliases=(),
                sim_require_finite=False, sim_require_nnan=False, nc=nc)
            return tuple(outs)

        devices = jax.devices()[:n_cores]
        mesh = Mesh(np.asarray(devices), ("core",))
        in_specs = (PartitionSpec("core"),) * (n_params + len(out_avals))
        out_specs = (PartitionSpec("core"),) * len(out_names)
        self.fn = jax.jit(
            shard_map(_body, mesh=mesh, in_specs=in_specs, out_specs=out_specs,
                      check_rep=False),
            donate_argnums=donate_idx, keep_unused=True)
        self._jax = jax

    def __call__(self, in_maps):
        per_core = [[np.asarray(m[n]) for n in self.in_names] for m in in_maps]
        concat_in = [np.concatenate([per_core[c][i] for c in range(self.n_cores)], axis=0)
                     for i in range(self.n_params)]
        concat_zeros = [np.zeros((self.n_cores * z.shape[0], *z.shape[1:]), z.dtype)
                        for z in self.zero_outs]
        out_arrs = self.fn(*concat_in, *concat_zeros)
        self._jax.block_until_ready(out_arrs)
        return [
            {n: np.asarray(out_arrs[i]).reshape(self.n_cores, *self.out_avals[i].shape)[c]
             for i, n in enumerate(self.out_names)}
            for c in range(self.n_cores)
        ]


def _get_runner(steps):
    if steps not in _RUNNER_CACHE:
        nc = _build(steps)
        _RUNNER_CACHE[steps] = _Runner(nc)
    return _RUNNER_CACHE[steps]


def kernel(hx, cx, W, U, L, steps):
    steps = int(steps)
    ins = _prep_inputs(hx, cx, W, U, L)
    runner = _get_runner(steps)
    res = runner(ins)
    out = np.concatenate([res[k]["out"] for k in range(NC)], axis=2)  # [steps, 64, 512]
    return out.astype(np.float32)

